# revision 108
# baseline (speedup 1.0000x reference)
"""Multi-head attention (B=1, n=4096, d=768, H=12) on 8 Trainium2 NeuronCores.

Sharding: 2 head-groups (6 heads = 384 dims) x 4 query-quarters (1024 q).
Core c = (hg = c // 4, sq = c % 4).

Per core:
  kT = Wk_hg @ K^T            [384, 4096]  (bf16, head-dim on partitions)
  qT = Wq_hg @ Q_sq^T         [384, 1024]
  v  = V @ Wv_hg^T (+ ones)   [4096, 6*65] (natural layout, 65th col = 1.0)
  S^T = k_h q_h^T  (pairs of heads row-packed on the PE, contraction dk=64)
  expS = exp(S^T / 8)   (ScalarE, PSUM->SBUF bf16, [128, 1024] per instr)
  o'^T_h[65, q] = [v_h | 1]^T @ expS    (row 64 = softmax denominators)
  oT_h = o'_h / sums  (reciprocal on DVE, broadcast across partitions by the
                       Pool engine's partition_broadcast, multiply on DVE —
                       no PE involvement)
  Y_part = oT^T @ WoT_hg      [1024, 768] fp32

Host: Y[sq] = part(hg=0, sq) + part(hg=1, sq) + bo.

Scheduling: the kernel is PE-bound overall (~241us of matmul at 2.4GHz vs
~199us of ScalarE exp), so the schedule keeps the PE stream dense from ~5us:
phase 0 starts after just one k-chunk + one q-chunk; the remaining mc=0
k-projection chunks, the full v projection, and the next q-chunk are PE
filler inside phase 0. Later phases each carry a balanced share of the
remaining k/q-projection chunks so no phase leaves the PE idle; softmax
normalization is deferred into the following phase and runs entirely on
Pool+DVE. Input DMAs are split across the SP/ACT/DVE queues so the first
score matmul has its operands at ~4us.
PSUM budget (8 banks): one shared triple-buffered pool (3x2 banks) serves
score tensors, projection chunks and the v projection; the other 2 banks
hold the two attnV accumulators.
"""

import numpy as np
import ml_dtypes

import concourse.bass as bass  # noqa: F401  (bass types used via tile/bacc)
import concourse.mybir as mybir
import concourse.tile as tile
from concourse import bacc
from concourse.bass_utils import run_bass_kernel_spmd

P = 128
D = 768
NPOS = 4096
NQ = 1024          # queries per core
KD = D // P        # 6 contraction tiles for projections
MC = 3             # 384 head-dims per group = 3 chunks of 128 (2 heads each)
NKT = NPOS // P    # 32 key-position tiles
DK = 64
VW = 65            # v columns per head incl. ones column
QCH = 512          # query chunk (one PSUM bank)
BF16 = mybir.dt.bfloat16
F32 = mybir.dt.float32
FP = mybir.ActivationFunctionType

_CACHED_NC = None
LAST_RESULTS = None  # BassKernelResults from the most recent run (for test.py)


def build_program():
    nc = bacc.Bacc("TRN2", target_bir_lowering=False, debug=False)

    KT = nc.dram_tensor("KT", [D, NPOS], BF16, kind="ExternalInput")
    VTb = nc.dram_tensor("VTb", [NKT, P, D], BF16, kind="ExternalInput")
    QT = nc.dram_tensor("QT", [D, NQ], BF16, kind="ExternalInput")
    WqT = nc.dram_tensor("WqT", [D, MC * P], BF16, kind="ExternalInput")
    WkT = nc.dram_tensor("WkT", [D, MC * P], BF16, kind="ExternalInput")
    WvT = nc.dram_tensor("WvT", [D, MC * P], BF16, kind="ExternalInput")
    WoT = nc.dram_tensor("WoT", [MC * P, D], BF16, kind="ExternalInput")
    BQK = nc.dram_tensor("BQK", [P, 2 * MC], F32, kind="ExternalInput")
    bvr = nc.dram_tensor("bvr", [P, MC * P], F32, kind="ExternalInput")
    Y = nc.dram_tensor("Y", [NQ, D], F32, kind="ExternalOutput")

    with tile.TileContext(nc) as tc:
        with (
            tc.tile_pool(name="const", bufs=1) as const,
            tc.tile_pool(name="persist", bufs=1) as persist,
            tc.tile_pool(name="vin", bufs=6) as vin,
            tc.tile_pool(name="expp", bufs=8) as expp,
            tc.tile_pool(name="small", bufs=3) as small,
            tc.tile_pool(name="ps_s", bufs=3, space="PSUM") as ps_s,
            tc.tile_pool(name="ps_o", bufs=2, space="PSUM") as ps_o,
        ):
            # ---- constants; DMAs split across the two HWDGE queues. The
            # scalar (ACT) queue only carries the small early tensors (its
            # sequencer must be free for the exp stream); K chunks and v
            # tiles stream on the sync queue, most of them emitted as JIT
            # callbacks inside phase 0 so they interleave by need-time. ----
            # Only the mc=0 slices of wk/wq gate the first projections; the
            # mc=1,2 slices are first used ~100us in and load late. K chunk 0
            # is split in half across both queues so the first k-projection
            # can start on its first 256 columns at ~5.5us.
            wk_sb = const.tile([P, KD, MC * P], BF16)
            WkT_r = WkT.rearrange("(k p) m -> p k m", p=P)
            nc.sync.dma_start(wk_sb[:, :, 0:P], WkT_r[:, :, 0:P])
            KT_res = persist.tile([P, KD, NPOS], BF16)
            KT_r = KT.rearrange("(k p) (t n) -> p k t n", p=P, n=QCH)
            nc.sync.dma_start(KT_res[:, :, 0:QCH], KT_r[:, :, 0])
            wv_sb = const.tile([P, KD, MC * P], BF16)
            nc.sync.dma_start(wv_sb, WvT.rearrange("(k p) m -> p k m", p=P))

            # scalar (ACT) queue: wq slice + QT first (the exp-stream
            # critical path), then the biases (needed only at the first DVE
            # add), then the late wk/wq slices
            wq_sb = const.tile([P, KD, MC * P], BF16)
            WqT_r = WqT.rearrange("(k p) m -> p k m", p=P)
            nc.scalar.dma_start(wq_sb[:, :, 0:P], WqT_r[:, :, 0:P])
            QT_res = persist.tile([P, KD, NQ], BF16)
            QT_r = QT.rearrange("(k p) (t n) -> p k t n", p=P, n=QCH)
            bqk_sb = const.tile([P, 2 * MC], F32)
            nc.scalar.dma_start(bqk_sb, BQK[:, :])
            nc.scalar.dma_start(QT_res[:, :, 0:QCH], QT_r[:, :, 0])
            nc.scalar.dma_start(QT_res[:, :, QCH:NQ], QT_r[:, :, 1])
            bvr_sb = const.tile([P, MC * P], F32)
            nc.scalar.dma_start(bvr_sb, bvr[:, :])

            def w_rest_load():
                nc.scalar.dma_start(wk_sb[:, :, P:MC * P],
                                    WkT_r[:, :, P:MC * P])
                nc.scalar.dma_start(wq_sb[:, :, P:MC * P],
                                    WqT_r[:, :, P:MC * P])

            ones_f32 = const.tile([1, DK], F32)
            nc.vector.memset(ones_f32, 1.0)
            ones_r = const.tile([1, DK], mybir.dt.float32r)
            with nc.allow_low_precision(reason="f32r ones for tail broadcast"):
                nc.vector.tensor_copy(ones_r, ones_f32)
            # trigger the exp table load while input DMAs stream
            warm_sb = const.tile([1, DK], F32)
            nc.scalar.activation(warm_sb, ones_f32, FP.Exp)

            wo_sb = const.tile([P, MC, D], BF16)

            def kload(nt):
                return lambda: nc.sync.dma_start(
                    KT_res[:, :, nt * QCH:(nt + 1) * QCH], KT_r[:, :, nt])

            def wo_load():
                nc.scalar.dma_start(wo_sb, WoT.rearrange("(k p) m -> p k m",
                                                         p=P))

            # ---- persistent activations ----
            kT_sb = persist.tile([P, MC, NPOS], BF16)
            qT_sb = persist.tile([P, MC, NQ], BF16)
            v_sb = persist.tile([P, NKT, 6 * VW], BF16)
            oT_sb = persist.tile([P, MC, NQ], BF16)

            # ones columns of v (65th col per head)
            v_heads = v_sb.rearrange("p m (h x) -> p m h x", x=VW)
            nc.vector.memset(v_heads[:, :, :, DK], 1.0)

            def proj_chunk(mc, w_sb, b_off, dst, n_total, nt, half=None,
                           _state={}):
                """One 512-wide projection chunk (6 accumulating matmuls +
                bias add). half=1/2 emits only the first/second 3 matmuls so
                the lump can straddle two kt steps of the exp stream without
                starving it of score tiles."""
                src = KT_res if n_total == NPOS else QT_res
                if half == 2:
                    ps = _state.pop((mc, nt, dst.name))
                    rng = range(KD // 2, KD)
                else:
                    ps = ps_s.tile([P, QCH], F32, tag="s2")
                    rng = range(KD // 2) if half == 1 else range(KD)
                    if half == 1:
                        _state[(mc, nt, dst.name)] = ps
                for kt in rng:
                    nc.tensor.matmul(
                        ps, w_sb[:, kt, mc * P:(mc + 1) * P],
                        src[:, kt, nt * QCH:(nt + 1) * QCH],
                        start=(kt == 0), stop=(kt == KD - 1),
                    )
                if half != 1:
                    nc.vector.tensor_scalar_add(
                        dst[:, mc, nt * QCH:(nt + 1) * QCH], ps,
                        bqk_sb[:, b_off + mc:b_off + mc + 1],
                    )

            v_tiles = {}

            def v_load(mt):
                vt = vin.tile([P, D], BF16, tag="vin")
                nc.sync.dma_start(vt, VTb[mt])
                v_tiles[mt] = vt

            def v_proj(mt):
                vt = v_tiles.pop(mt)
                ps = ps_s.tile([P, QCH], F32, tag="s2")
                for kt in range(KD):
                    nc.tensor.matmul(
                        ps[:, 0:MC * P], vt[:, kt * P:(kt + 1) * P],
                        wv_sb[:, kt, :],
                        start=(kt == 0), stop=(kt == KD - 1),
                    )
                nc.vector.tensor_tensor(
                    v_heads[:, mt, :, 0:DK],
                    ps[:, 0:MC * P].rearrange("p (h x) -> p h x", x=DK),
                    bvr_sb.rearrange("p (h x) -> p h x", x=DK),
                    mybir.AluOpType.add,
                )

            def scores(mc, qs, kt):
                s2 = ps_s.tile([P, 2, QCH], F32, tag="s2")
                nc.tensor.matmul(
                    s2[:, 0, :], kT_sb[0:DK, mc, kt * P:(kt + 1) * P],
                    qT_sb[0:DK, mc, qs], start=True, stop=True,
                )
                nc.tensor.matmul(
                    s2[:, 1, :], kT_sb[DK:P, mc, kt * P:(kt + 1) * P],
                    qT_sb[DK:P, mc, qs], start=True, stop=True,
                )
                return s2

            def kchunk00():
                """Startup k-projection chunk, column-halved so it can begin
                on the first 256 columns of K chunk 0 while the second half
                is still in flight on the other queue."""
                ps = ps_s.tile([P, 2, QCH // 2], F32, tag="s2")
                for h in range(2):
                    cs = slice(h * (QCH // 2), (h + 1) * (QCH // 2))
                    for kt in range(KD):
                        nc.tensor.matmul(
                            ps[:, h, :], wk_sb[:, kt, 0:P], KT_res[:, kt, cs],
                            start=(kt == 0), stop=(kt == KD - 1),
                        )
                nc.vector.tensor_scalar_add(
                    kT_sb[:, 0, 0:QCH], ps.rearrange("p a b -> p (a b)"),
                    bqk_sb[:, MC:MC + 1],
                )

            def out_proj(qt, n0_only=None):
                """Output projection of one 128-query tile; n0_only emits a
                single 512- or 256-column pass so the two passes can sit at
                different key-tiles of the exp stream."""
                y_sb = small.tile([P, D], F32, tag="y")
                for n0, nsz in ((0, 512), (512, 256)):
                    if n0_only is not None and n0 != n0_only:
                        continue
                    ps = ps_s.tile([P, QCH], F32, tag="s2")
                    for mc in range(MC):
                        nc.tensor.matmul(
                            ps[:, 0:nsz], oT_sb[:, mc, qt * P:(qt + 1) * P],
                            wo_sb[:, mc, n0:n0 + nsz],
                            start=(mc == 0), stop=(mc == MC - 1),
                        )
                    nc.vector.tensor_copy(y_sb[:, n0:n0 + nsz], ps[:, 0:nsz])
                    nc.sync.dma_start(Y[qt * P:(qt + 1) * P, n0:n0 + nsz],
                                      y_sb[:, n0:n0 + nsz])

            # out-projection of qt 4..7, split by contraction: the mc=0,1
            # partials are late-phase-5 PE filler (keeping the PE from
            # running ahead of ACT into the 4-deep wait-queue block); only
            # the mc=2 pass + add remain exposed after the final normalize.
            y_part = persist.tile([P, 4, D], F32)

            def oproj_partial(qt, n0_only=None):
                for n0, nsz in ((0, 512), (512, 256)):
                    if n0_only is not None and n0 != n0_only:
                        continue
                    ps = ps_s.tile([P, QCH], F32, tag="s2")
                    for mc in range(2):
                        nc.tensor.matmul(
                            ps[:, 0:nsz], oT_sb[:, mc, qt * P:(qt + 1) * P],
                            wo_sb[:, mc, n0:n0 + nsz],
                            start=(mc == 0), stop=(mc == 1),
                        )
                    nc.vector.tensor_copy(y_part[:, qt - 4, n0:n0 + nsz],
                                          ps[:, 0:nsz])

            def oproj_final(qt):
                eng = nc.vector
                y_sb = small.tile([P, D], F32, tag="y")
                for n0, nsz in ((0, 512), (512, 256)):
                    ps = ps_s.tile([P, QCH], F32, tag="s2")
                    nc.tensor.matmul(
                        ps[:, 0:nsz], oT_sb[:, 2, qt * P:(qt + 1) * P],
                        wo_sb[:, 2, n0:n0 + nsz], start=True, stop=True,
                    )
                    eng.tensor_tensor(
                        y_sb[:, n0:n0 + nsz], ps[:, 0:nsz],
                        y_part[:, qt - 4, n0:n0 + nsz], mybir.AluOpType.add,
                    )
                    nc.sync.dma_start(Y[qt * P:(qt + 1) * P, n0:n0 + nsz],
                                      y_sb[:, n0:n0 + nsz])

            def normalize(mc, qc, oc_pair, use_pe=False):
                """Deferred: divide o' by the softmax sums, write oT_sb.

                Reciprocal on DVE, partition-broadcast on Pool (or, in the
                exposed tail where the PE is idle, a 1-row f32r ones-matmul
                which has less latency than the Q7 path), multiply on DVE.
                """
                qs = slice(qc * QCH, (qc + 1) * QCH)
                # upper-half head first: its oT write needs a partition-shift
                # DMA, which then overlaps the lower half's DVE work
                for idx, oc in ((1, oc_pair[1]), (0, oc_pair[0])):
                    if use_pe:
                        r = small.tile([1, QCH], mybir.dt.float32r, tag="rf")
                        with nc.allow_low_precision(
                                reason="f32r reciprocal for PE broadcast"):
                            nc.vector.reciprocal(r, oc[DK:DK + 1, :])
                        rr_ps = ps_s.tile([DK, QCH], F32, tag="s2")
                        nc.tensor.matmul(rr_ps, ones_r, r, start=True,
                                         stop=True)
                        rr = small.tile([DK, QCH], F32, tag="rr")
                        nc.vector.tensor_copy(rr, rr_ps)
                    else:
                        r = small.tile([1, QCH], F32, tag="r")
                        nc.vector.reciprocal(r, oc[DK:DK + 1, :])
                        rr = small.tile([DK, QCH], F32, tag="rr")
                        nc.gpsimd.partition_broadcast(rr[:, :], r[:, :])
                    if idx == 0:
                        nc.vector.tensor_tensor(
                            oT_sb[0:DK, mc, qs], oc[0:DK, :], rr,
                            mybir.AluOpType.mult,
                        )
                    else:
                        ob = small.tile([DK, QCH], BF16, tag="ob")
                        nc.vector.tensor_tensor(
                            ob, oc[0:DK, :], rr, mybir.AluOpType.mult,
                        )
                        nc.sync.dma_start(oT_sb[DK:P, mc, qs], ob)

            def attn(mc, qc, jit_work=None, defer_norm=True, carry=None):
                """Attention for head pair mc over query chunk qc.

                jit_work: {kt: [callbacks]} — PE work emitted at exactly
                iteration kt (just-in-time k/q/v chunks, spread filler).
                defer_norm: return a finish closure (attnV flush + copies,
                run at the NEXT phase's start via carry=) which itself
                returns the normalize closure; if False, flush+normalize
                straight from PSUM (shorter chain — final phase).
                carry: the previous phase's finish closure; emitted right
                after this phase's score prefetch so the exp stream never
                waits on the transition.
                """
                qs = slice(qc * QCH, (qc + 1) * QCH)
                jit = jit_work or {}
                assert all(0 <= kt < NKT for kt in jit), sorted(jit)
                oA = ps_o.tile([VW, QCH], F32, tag="o")
                oB = ps_o.tile([VW, QCH], F32, tag="o")
                # scores run two key-tiles ahead of exp (3-deep psum rotation)
                s2q = [scores(mc, qs, 0), scores(mc, qs, 1)]
                if carry is not None:
                    carry()

                def attnv(k2, e2):
                    nc.tensor.matmul(
                        oA, v_sb[:, k2, (2 * mc) * VW:(2 * mc) * VW + VW],
                        e2[:, 0, :],
                        start=(k2 == 0), stop=(k2 == NKT - 1),
                    )
                    nc.tensor.matmul(
                        oB, v_sb[:, k2, (2 * mc + 1) * VW:(2 * mc + 1) * VW + VW],
                        e2[:, 1, :],
                        start=(k2 == 0), stop=(k2 == NKT - 1),
                    )

                # attnV lags the exp stream by 2 key-tiles so the PE never
                # reaches an attnV before its e-tile is ready (the expp pool
                # covers the extra liveness); no lag in the final phase,
                # where the flush would extend the exposed tail
                lag = 2 if defer_norm else 1
                pend = []
                for kt in range(NKT):
                    e = expp.tile([P, 2, QCH], BF16, tag="e")
                    nc.scalar.activation(e, s2q.pop(0), FP.Exp, scale=0.125)
                    if kt + 2 < NKT:
                        s2q.append(scores(mc, qs, kt + 2))
                    pend.append((kt, e))
                    for cb in jit.get(kt, ()):
                        cb()
                    if len(pend) > lag:
                        attnv(*pend.pop(0))
                if not defer_norm:
                    for item in pend:
                        attnv(*item)
                    normalize(mc, qc, (oA, oB))
                    return None

                def finish():
                    for item in pend:
                        attnv(*item)
                    # free the o-psum banks: copy to SBUF, normalize later
                    ocs = []
                    for o in (oA, oB):
                        oc = small.tile([VW, QCH], F32, tag="oc")
                        nc.vector.tensor_copy(oc, o)
                        ocs.append(oc)
                    return lambda: normalize(mc, qc, ocs)

                return finish

            # ---- emission order: dense PE stream from the first exp on ----
            def kchunk(mc, nt, half=None):
                return lambda: proj_chunk(mc, wk_sb, MC, kT_sb, NPOS, nt,
                                          half)

            def qchunk(mc, nt, half=None):
                return lambda: proj_chunk(mc, wq_sb, 0, qT_sb, NQ, nt,
                                          half)

            def add_jit(jit, kt, cb):
                jit.setdefault(kt, []).append(cb)

            def add_jit_halved(jit, kt, chunk_fn, mc, nt):
                add_jit(jit, kt, chunk_fn(mc, nt, half=1))
                add_jit(jit, kt + 1, chunk_fn(mc, nt, half=2))

            # startup: just the first k-chunk and q-chunk — everything else
            # is JIT filler inside the phases. v tiles prefetch ~3 ahead.
            kchunk(0, 0)()
            v_load(0)
            kload(1)()
            v_load(1)
            v_load(2)
            qchunk(0, 0)()

            # phase 0 = attn(0,0): v projection 1 tile ahead (loads 3 ahead),
            # K-chunk DMAs at kt=4*nt-7, the remaining mc=0 k-chunks at
            # kt=4*nt-3, and the next q-chunk at the end.
            jit0 = {}
            v_proj(0)
            for kt in range(NKT - 1):
                if kt + 3 < NKT:
                    add_jit(jit0, kt, (lambda m=kt + 3: v_load(m)))
                add_jit(jit0, kt, (lambda m=kt + 1: v_proj(m)))
            for nt in range(2, NPOS // QCH):
                add_jit(jit0, 4 * nt - 7, kload(nt))
            for nt in range(1, NPOS // QCH):
                add_jit(jit0, 4 * nt - 3, kchunk(0, nt))
            add_jit(jit0, 6, w_rest_load)
            add_jit(jit0, 20, wo_load)
            add_jit(jit0, 29, qchunk(0, 1))
            f00 = attn(0, 0, jit0)

            # phase 1 = attn(0,1): kT1 chunks 0..3, qT1 chunk 0, norm(0,0)
            jit1 = {}
            for nt in range(4):
                add_jit_halved(jit1, 6 * nt + 4, kchunk, 1, nt)
            cell1 = {}
            add_jit(jit1, 3, lambda: cell1["n"]())
            add_jit_halved(jit1, 27, qchunk, 1, 0)
            f01 = attn(0, 1, jit1, carry=lambda: cell1.update(n=f00()))

            # phase 2 = attn(1,0): kT1 chunks 4..7 JIT, qT1 chunk 1, norm(0,1)
            jit2 = {}
            for nt in range(4, NPOS // QCH):
                add_jit_halved(jit2, 5 * nt - 15, kchunk, 1, nt)
            cell2 = {}
            add_jit(jit2, 3, lambda: cell2["n"]())
            add_jit_halved(jit2, 23, qchunk, 1, 1)
            f10 = attn(1, 0, jit2, carry=lambda: cell2.update(n=f01()))

            # phase 3 = attn(1,1): kT2 chunks 0..3, qT2 chunk 0, norm(1,0)
            jit3 = {}
            for nt in range(4):
                add_jit_halved(jit3, 6 * nt + 4, kchunk, 2, nt)
            cell3 = {}
            add_jit(jit3, 3, lambda: cell3["n"]())
            add_jit_halved(jit3, 27, qchunk, 2, 0)
            f11 = attn(1, 1, jit3, carry=lambda: cell3.update(n=f10()))

            # phase 4 = attn(2,0): kT2 chunks 4..7 JIT, qT2 chunk 1, norm(1,1)
            jit4 = {}
            for nt in range(4, NPOS // QCH):
                add_jit_halved(jit4, 5 * nt - 15, kchunk, 2, nt)
            cell4 = {}
            add_jit(jit4, 3, lambda: cell4["n"]())
            add_jit_halved(jit4, 23, qchunk, 2, 1)
            f20 = attn(2, 0, jit4, carry=lambda: cell4.update(n=f11()))

            # phase 5 = attn(2,1): norm(2,0), out-projection of query half 0,
            # then the mc=0,1 partials of query half 1 late in the phase
            jit5 = {}
            cell5 = {}
            add_jit(jit5, 1, lambda: cell5["n"]())
            for qt in range(4):
                add_jit(jit5, 7 + 4 * qt, (lambda q=qt: out_proj(q, 0)))
                add_jit(jit5, 9 + 4 * qt, (lambda q=qt: out_proj(q, 512)))
            for qt in range(4, NQ // P):
                add_jit(jit5, 21 + 3 * (qt - 4),
                        (lambda q=qt: oproj_partial(q, 0)))
                add_jit(jit5, 22 + 3 * (qt - 4),
                        (lambda q=qt: oproj_partial(q, 512)))
            attn(2, 1, jit5, defer_norm=False,
                 carry=lambda: cell5.update(n=f20()))

            # keep the PE p-state ramped through the final-normalize window:
            # a contiguous stream of tiny scratch matmuls (never read back)
            # bridges the ~5us idle that would otherwise reset the clock
            # ramp and slow the tail out-projections ~3.7x
            warm_ps = ps_s.tile([P, QCH], F32, tag="s2")
            for i in range(80):
                nc.tensor.matmul(warm_ps[0:DK, 0:DK], wo_sb[:, 0, 0:DK],
                                 oT_sb[:, 0, 0:DK],
                                 start=(i == 0), stop=(i == 79))

            for qt in range(4, NQ // P):
                oproj_final(qt)

    nc.compile()
    return nc


def kernel(**inputs):
    global _CACHED_NC, LAST_RESULTS
    bf = ml_dtypes.bfloat16
    f32 = np.float32

    Q = np.asarray(inputs["Q"], dtype=f32)
    K = np.asarray(inputs["K"], dtype=f32)
    V = np.asarray(inputs["V"], dtype=f32)
    Wq = np.asarray(inputs["Wq"], dtype=f32)
    bq = np.asarray(inputs["bq"], dtype=f32)
    Wk = np.asarray(inputs["Wk"], dtype=f32)
    bk = np.asarray(inputs["bk"], dtype=f32)
    Wv = np.asarray(inputs["Wv"], dtype=f32)
    bv = np.asarray(inputs["bv"], dtype=f32)
    Wo = np.asarray(inputs["Wo"], dtype=f32)
    bo = np.asarray(inputs["bo"], dtype=f32)

    KTh = np.ascontiguousarray(K[0].T).astype(bf)                 # [768, 4096]
    VT = V[0].T                                                   # [768, 4096]
    VTb = np.ascontiguousarray(
        VT.reshape(KD, P, NKT, P).transpose(2, 1, 0, 3).reshape(NKT, P, D)
    ).astype(bf)
    QTs = [
        np.ascontiguousarray(Q[0, sq * NQ:(sq + 1) * NQ, :].T).astype(bf)
        for sq in range(4)
    ]

    per_hg = []
    for hg in range(2):
        sl = slice(hg * 384, (hg + 1) * 384)
        per_hg.append(dict(
            WqT=np.ascontiguousarray(Wq[sl, :].T).astype(bf),
            WkT=np.ascontiguousarray(Wk[sl, :].T).astype(bf),
            WvT=np.ascontiguousarray(Wv[sl, :].T).astype(bf),
            WoT=np.ascontiguousarray(Wo[:, sl].T).astype(bf),
            BQK=np.ascontiguousarray(np.concatenate(
                [bq[sl].reshape(MC, P).T, bk[sl].reshape(MC, P).T], axis=1,
            )).astype(f32),
            bvr=np.ascontiguousarray(
                np.broadcast_to(bv[sl][None, :], (P, 384))
            ).astype(f32),
        ))

    in_maps = []
    for c in range(8):
        hg, sq = c // 4, c % 4
        in_maps.append(dict(
            KT=KTh, VTb=VTb, QT=QTs[sq], **per_hg[hg],
        ))

    if _CACHED_NC is None:
        _CACHED_NC = build_program()
    nc = _CACHED_NC

    LAST_RESULTS = run_bass_kernel_spmd(nc, in_maps, core_ids=list(range(8)))
    parts = [r["Y"] for r in LAST_RESULTS.results]

    out = np.empty((1, NPOS, D), dtype=f32)
    for sq in range(4):
        out[0, sq * NQ:(sq + 1) * NQ] = parts[sq] + parts[4 + sq] + bo[None, :]
    return out
